# revision 37
# baseline (speedup 1.0000x reference)
"""Multi-head self-attention TRN2 Bass kernel.

Problem: B=4, S=2048, EMB=1024, H=16 heads, dqk=dv=64.
Sharding: 8 cores; core c handles batch b=c//2 and head group g=c%2
(8 heads each). Each core computes its partial output projection
(rows of wo for its heads); host sums the two partials per batch and
adds bo.

Key optimizations over the straightforward version:
  - Q/K projections run as fp8e4 DoubleRow matmuls (256-deep
    contraction per instruction, 2x FLOP rate). Host pre-packs x/w in
    the [128p, 2k, .] interleave and scales weights+biases by 16 to
    avoid fp8 denormals; the 1/256 compensation folds into the exp
    scale (0.125/256).
  - Scores matmuls for the two heads of a pair are emitted
    interleaved at base partitions 0/64 so the K=64 matmuls row-tile
    into concurrent execution on the PE (measured ~2.1x).
  - V projection t-major with a ones column per head block so the AV
    matmul emits softmax denominators for free (row 64 of its out).
  - exp on ACT reads scores PSUM [128,1024] tiles (max-free softmax;
    |scores| small), writes bf16 P^T.
  - AV accumulates Z~T [66, 512] over 16 t-tiles in PSUM.
  - normalize: reciprocal of D row, DMA round-trip broadcast across
    partitions, multiply + bv bias -> ZnormT [512 j, 2048 s] bf16.
  - out projection bf16 -> partial OUT [2048, 1024] f32 -> DRAM.
"""

import ml_dtypes
import numpy as np

import concourse.bass as bass
import concourse.tile as tile
from concourse import bacc, mybir
from concourse.bass_utils import run_bass_kernel_spmd

B, S, EMB, H, DH = 4, 2048, 1024, 16, 64
N_CORES = 8
HPC = H // 2          # heads per core
JC = HPC * DH         # 512: per-core projected width
VB = DH + 2           # 66: per-head V block (64 V cols + ones + pad)
WSCALE = 16.0         # fp8 weight pre-scale (host); folded into exp scale

F32 = mybir.dt.float32
BF16 = mybir.dt.bfloat16
FP8 = mybir.dt.float8e4


def build_kernel(reps=1, mode='full'):
    nc = bacc.Bacc(
        "TRN2", target_bir_lowering=False, debug=False, num_devices=N_CORES
    )

    # fp8 DoubleRow operands for Q/K proj (e' = blk*256 + k*128 + p order,
    # which is just the natural row order)
    xq8 = nc.dram_tensor("xq8", [EMB, S], FP8, kind="ExternalInput").ap()
    xk8 = nc.dram_tensor("xk8", [EMB, S], FP8, kind="ExternalInput").ap()
    wq8_d = nc.dram_tensor("wq8", [EMB, JC], FP8, kind="ExternalInput").ap()
    wk8_d = nc.dram_tensor("wk8", [EMB, JC], FP8, kind="ExternalInput").ap()
    # bf16 operands for V proj
    xkv = nc.dram_tensor("xkv", [EMB, S], BF16, kind="ExternalInput").ap()
    wv_d = nc.dram_tensor("wv", [EMB, JC], BF16, kind="ExternalInput").ap()
    bq_d = nc.dram_tensor("bq", [JC], F32, kind="ExternalInput").ap()
    bk_d = nc.dram_tensor("bk", [JC], F32, kind="ExternalInput").ap()
    bv_d = nc.dram_tensor("bv", [JC], F32, kind="ExternalInput").ap()
    wo_d = nc.dram_tensor("wo", [JC, EMB], BF16, kind="ExternalInput").ap()
    out_d = nc.dram_tensor("out", [S, EMB], F32, kind="ExternalOutput").ap()
    dr_d = nc.dram_tensor("dr_scratch", [2, S], BF16).ap()  # Drecip bounce

    import contextlib

    with tile.TileContext(nc) as tc:
        with (
            tc.For_i(0, reps, 1) if reps > 1 else contextlib.nullcontext(),
            tc.tile_pool(name="persist", bufs=1) as pp,
        ):
            # persistent SBUF tensors
            qht = [pp.tile([128, S], BF16, name=f"qht{i}") for i in range(4)]
            kht = [pp.tile([128, S], BF16, name=f"kht{i}") for i in range(4)]
            vh = [pp.tile([128, HPC * VB], BF16, name=f"vh{t}")
                  for t in range(16)]
            znorm = [pp.tile([128, S], BF16, name=f"zn{i}") for i in range(4)]
            wos = [pp.tile([128, EMB], BF16, name=f"wo{j}") for j in range(4)]
            bias_q = pp.tile([128, 4], F32, name="bias_q")
            bias_k = pp.tile([128, 4], F32, name="bias_k")
            bias_v = pp.tile([64, HPC], F32, name="bias_v")  # [d, head]

            nc.sync.dma_start(bias_q[:], bq_d.rearrange("(c p) -> p c", p=128))
            nc.sync.dma_start(bias_k[:], bk_d.rearrange("(c p) -> p c", p=128))
            # ones columns in vh blocks (col 64 of each 66-block); pad col 0
            for t in range(16):
                blocks = vh[t][:].rearrange("p (h c) -> p h c", c=VB)
                nc.vector.memset(blocks[:, :, DH:DH + 1], 1.0)
                nc.vector.memset(blocks[:, :, DH + 1:], 0.0)

            # ------------- fused projections + attention -------------
            # pair-0 QK proj gates the first exp; V-proj uses the av PSUM
            # slots so pair-0 scores/exp (sp slots) start immediately
            # after. Pairs 1-3 QK proj is emitted inside the previous
            # pair's attention t-loop so its sp-ring slots interleave
            # with the scores pipeline instead of all queuing ahead of
            # it (tile-tag rings are ordered by emission).
            with (
                tc.tile_pool(name="sps", bufs=1, space="PSUM") as sp_pool,
                tc.tile_pool(name="avps", bufs=1, space="PSUM") as av_pool,
                tc.tile_pool(name="dreb_p", bufs=2) as dre_pool,
                tc.tile_pool(name="znsc", bufs=1) as zns_pool,
                tc.tile_pool(name="drec_p", bufs=1) as drec_pool,
                tc.tile_pool(name="avstg", bufs=2) as stg_pool,
                tc.tile_pool(name="xr_p", bufs=1) as xr_pool,
                tc.tile_pool(name="wqkv", bufs=1) as w_pool,
            ):
                # fp8 DoubleRow tiles: [128p, 2k, .] with k (the 128-row
                # subtile index) major in the free dim
                wq8 = [w_pool.tile([128, 2 * JC], FP8, name=f"wq8_{e}")
                       for e in range(4)]
                wk8 = [w_pool.tile([128, 2 * JC], FP8, name=f"wk8_{e}")
                       for e in range(4)]
                wvs = [w_pool.tile([128, JC], BF16, name=f"wvs{e}")
                       for e in range(8)]
                xq8r = [xr_pool.tile([128, 2 * S], FP8, name=f"xq8r{e}")
                        for e in range(4)]
                xk8r = [xr_pool.tile([128, 2 * S], FP8, name=f"xk8r{e}")
                        for e in range(4)]
                xkr = [xr_pool.tile([128, S], BF16, name=f"xkr{e}")
                       for e in range(8)]
                # DMA in first-use order: Q/K fp8 path first (gates the
                # first exp), then the V-proj operands, then wo/bias_v
                # (needed last).
                for e in range(4):
                    for k in range(2):
                        r0 = e * 256 + k * 128
                        nc.sync.dma_start(
                            wq8[e][:, k * JC:(k + 1) * JC],
                            wq8_d[r0:r0 + 128, :])
                half = S // 2
                for e in range(4):
                    for k in range(2):
                        r0 = e * 256 + k * 128
                        nc.sync.dma_start(
                            xq8r[e][:, k * S:k * S + half],
                            xq8[r0:r0 + 128, 0:half])
                for e in range(4):
                    for k in range(2):
                        r0 = e * 256 + k * 128
                        nc.sync.dma_start(
                            wk8[e][:, k * JC:(k + 1) * JC],
                            wk8_d[r0:r0 + 128, :])
                for e in range(4):
                    for k in range(2):
                        r0 = e * 256 + k * 128
                        nc.sync.dma_start(
                            xk8r[e][:, k * S:k * S + half],
                            xk8[r0:r0 + 128, 0:half])
                for xr_t, xd in ((xq8r, xq8), (xk8r, xk8)):
                    for e in range(4):
                        for k in range(2):
                            r0 = e * 256 + k * 128
                            nc.sync.dma_start(
                                xr_t[e][:, k * S + half:(k + 1) * S],
                                xd[r0:r0 + 128, half:S])
                for e in range(8):
                    nc.sync.dma_start(wvs[e][:],
                                      wv_d[e * 128:(e + 1) * 128, :])
                for e in range(8):
                    nc.sync.dma_start(xkr[e][:],
                                      xkv[e * 128:(e + 1) * 128, :])
                nc.sync.dma_start(bias_v[:],
                                  bv_d.rearrange("(h d) -> d h", d=DH))
                for j in range(4):
                    nc.sync.dma_start(wos[j][:],
                                      wo_d[j * 128:(j + 1) * 128, :])

                # sc-interleaved so scores/exp for s_half 0 can start
                # after Q sc0,sc1 + K sc0
                PROJ_ORDER = (('q', 0), ('q', 1), ('k', 0), ('k', 1),
                              ('q', 2), ('k', 2), ('q', 3), ('k', 3))

                def proj_one(pair, which, sc):
                    jsl = slice(pair * 128, (pair + 1) * 128)
                    dst, ws, xr, bias = (
                        (qht, wq8, xq8r, bias_q) if which == 'q'
                        else (kht, wk8, xk8r, bias_k))
                    s0 = sc * 512
                    ps = sp_pool.tile([128, 512], F32, tag=f"sp{sc % 2}",
                                      name=f"sp{sc % 2}")
                    for e in range(4):
                        w3 = ws[e][:].rearrange("p (k j) -> p k j", k=2)
                        x3 = xr[e][:].rearrange("p (k s) -> p k s", k=2)
                        nc.tensor.matmul(
                            ps[:],
                            w3[:, :, jsl],
                            x3[:, :, s0:s0 + 512],
                            start=(e == 0), stop=(e == 3),
                            perf_mode=mybir.MatmulPerfMode.DoubleRow,
                        )
                    nc.vector.tensor_scalar_add(
                        dst[pair][:, s0:s0 + 512], ps[:],
                        bias[:, pair:pair + 1]
                    )

                def emit_vproj():
                    # V-proj goes through the av PSUM slots (free until
                    # the first AV accumulation) so it can overlap with
                    # pair-0 scores+exp, which use the sp slots.
                    for tch in range(16):
                        tsl = slice(tch * 128, (tch + 1) * 128)
                        vt = tch % 4
                        ps = av_pool.tile(
                            [128, 512], F32,
                            tag=f"av{vt // 2}{vt % 2}",
                            name=f"av{vt // 2}{vt % 2}")
                        for e in range(8):
                            nc.tensor.matmul(
                                ps[:], xkr[e][:, tsl], wvs[e][:],
                                start=(e == 0), stop=(e == 7),
                            )
                        nc.vector.tensor_copy(
                            vh[tch][:].rearrange(
                                "p (h c) -> p h c", c=VB)[:, :, 0:DH],
                            ps[:].rearrange("p (h d) -> p h d", d=DH),
                        )

                def emit_attn(pair, pt_pool, inject_proj_pair=None):
                    for s_half in range(2):
                        s0 = s_half * 1024
                        avs = {}
                        for par in range(2):
                            for i in range(2):
                                avs[(par, i)] = av_pool.tile(
                                    [VB, 512], F32, tag=f"av{par}{i}",
                                    name=f"av{par}{i}")

                        def emit_scores(t):
                            """Both heads' score matmuls interleaved so the
                            K=64 pairs row-tile concurrently (base
                            partitions 0 / 64)."""
                            sps = []
                            for par in range(2):
                                sps.append(sp_pool.tile(
                                    [128, 1024], F32, tag=f"sp{par}",
                                    name=f"sp{par}"))
                            for rep in range(2 if mode == 'sc2' else 1):
                                for i in range(2):
                                    for par in range(2):
                                        off = par * 64
                                        nc.tensor.matmul(
                                            sps[par][:,
                                                     i * 512:(i + 1) * 512],
                                            kht[pair][off:off + 64,
                                                      t * 128:(t + 1) * 128],
                                            qht[pair][off:off + 64,
                                                      s0 + i * 512:
                                                      s0 + (i + 1) * 512],
                                            start=True,
                                            stop=True,
                                        )
                            ptts = []
                            for par in range(2):
                                ptt = pt_pool.tile([128, 1024], BF16,
                                                   tag=f"ptt{par}",
                                                   name=f"ptt{par}")
                                nc.scalar.activation(
                                    ptt[:], sps[par][:],
                                    mybir.ActivationFunctionType.Exp,
                                    scale=0.125 / (WSCALE * WSCALE),
                                )
                                if mode == 'exp2':
                                    pt2 = pt_pool.tile(
                                        [128, 1024], BF16,
                                        tag=f"pt2{par}", name=f"pt2{par}")
                                    nc.scalar.activation(
                                        pt2[:], sps[par][:],
                                        mybir.ActivationFunctionType.Exp,
                                        scale=0.125 / (WSCALE * WSCALE),
                                    )
                                ptts.append(ptt)
                            return ptts

                        def emit_av(t, par, ptt):
                            h = pair * 2 + par
                            for i in range(2):
                                for rep in range(2 if mode == 'av2' else 1):
                                    nc.tensor.matmul(
                                        avs[(par, i)][:],
                                        vh[t][:, h * VB:(h + 1) * VB],
                                        ptt[:, i * 512:(i + 1) * 512],
                                        start=(t == 0 and rep == 0),
                                        stop=(t == 15 and
                                              rep == (1 if mode == 'av2'
                                                      else 0)),
                                        skip_group_check=True,
                                    )

                        prev = None
                        for t in range(16):
                            ptts = emit_scores(t)
                            if prev is not None:
                                emit_av(prev[0], 0, prev[1])
                                emit_av(prev[0], 1, prev[2])
                            if (inject_proj_pair is not None
                                    and s_half == 1 and t % 2 == 0):
                                which, sc = PROJ_ORDER[t // 2]
                                proj_one(inject_proj_pair, which, sc)
                            prev = (t, ptts[0], ptts[1])
                        emit_av(prev[0], 0, prev[1])
                        emit_av(prev[0], 1, prev[2])

                        # stage avs out to SBUF fast so the next s_half's
                        # AV t=0 only waits on these copies, not on the
                        # full normalize chain
                        stg = {}
                        for par in range(2):
                            for i in range(2):
                                stg[(par, i)] = stg_pool.tile(
                                    [66, 512], BF16, tag=f"avs{par}{i}",
                                    name=f"avs{par}{i}")
                                nc.vector.tensor_copy(
                                    stg[(par, i)][:], avs[(par, i)][:])
                        for par in range(2):
                            h = pair * 2 + par
                            off = par * 64
                            dreb = dre_pool.tile(
                                [64, 1024], BF16, tag=f"dreb{par}",
                                name=f"dreb{par}")
                            for i in range(2):
                                drc = drec_pool.tile(
                                    [1, 512], BF16, tag=f"drc{par}",
                                    name=f"drc{par}")
                                with nc.allow_low_precision(
                                        reason="1/D in bf16: recip err "
                                        "~0.2% is shared per-head and "
                                        "attenuates to ~3e-5 in out"):
                                    nc.vector.reciprocal(
                                        drc[0:1, :],
                                        stg[(par, i)][DH:DH + 1, :],
                                    )
                                nc.sync.dma_start(
                                    dr_d[par,
                                         s0 + i * 512:s0 + (i + 1) * 512],
                                    drc[0:1, :],
                                )
                            nc.sync.dma_start(
                                dreb[:],
                                dr_d[par:par + 1, s0:s0 + 1024]
                                .broadcast_to([64, 1024]),
                            )
                            zn_s = zns_pool.tile(
                                [64, 1024], BF16, tag=f"zn_s{par}",
                                name=f"zn_s{par}")
                            for i in range(2):
                                nc.vector.tensor_mul(
                                    zn_s[:, i * 512:(i + 1) * 512],
                                    stg[(par, i)][0:DH, :],
                                    dreb[:, i * 512:(i + 1) * 512],
                                )
                            nc.vector.tensor_scalar_add(
                                zn_s[:], zn_s[:], bias_v[:, h:h + 1]
                            )
                            nc.sync.dma_start(
                                znorm[pair][off:off + 64, s0:s0 + 1024],
                                zn_s[:],
                            )

                with tc.tile_pool(name="pt", bufs=8) as pt_pool:
                    n_pairs = {'pairs1': 1, 'pairs0': 0}.get(mode, 4)
                    for p_ in range(n_pairs, 4):
                        nc.vector.memset(znorm[p_][:], 0.0)
                    for idx in range(8):
                        proj_one(0, *PROJ_ORDER[idx])
                    emit_vproj()
                    for pair in range(n_pairs):
                        emit_attn(pair, pt_pool,
                                  inject_proj_pair=(pair + 1
                                                    if pair < 3 else None))

            # ---------------- output projection ----------------
            with (
                tc.tile_pool(name="ops", bufs=4, space="PSUM") as op_pool,
                tc.tile_pool(name="ostg", bufs=4) as ostg_pool,
            ):
                for scc in range(16):
                    psl = slice(scc * 128, (scc + 1) * 128)
                    for oc in range(2):
                        ps = op_pool.tile([128, 512], F32, tag="ops",
                                          name="ops")
                        osl = slice(oc * 512, (oc + 1) * 512)
                        for jt in range(4):
                            nc.tensor.matmul(
                                ps[:],
                                znorm[jt][:, psl],
                                wos[jt][:, osl],
                                start=(jt == 0),
                                stop=(jt == 3),
                            )
                        ostg = ostg_pool.tile([128, 512], F32, tag="ostg",
                                              name="ostg")
                        nc.vector.tensor_copy(ostg[:], ps[:])
                        nc.sync.dma_start(out_d[psl, osl], ostg[:])

    nc.compile()
    return nc


def _bf16(a):
    return np.asarray(a, np.float32).astype(ml_dtypes.bfloat16)


def _fp8(a):
    return np.asarray(a, np.float32).astype(ml_dtypes.float8_e4m3fn)


def _prep_inputs(q, k_and_v, wq, bq, wk, bk, wv, bv, wo):
    """Build per-core input maps."""
    in_maps = []
    for c in range(N_CORES):
        b, g = c // 2, c % 2
        hs = slice(g * HPC, (g + 1) * HPC)
        # [H, emb, d] -> [emb, H*d] for this head group
        wq_g = np.transpose(wq[hs], (1, 0, 2)).reshape(EMB, JC)
        wk_g = np.transpose(wk[hs], (1, 0, 2)).reshape(EMB, JC)
        wv_g = np.transpose(wv[hs], (1, 0, 2)).reshape(EMB, JC)
        xq_t = np.ascontiguousarray(np.asarray(q[b], np.float32).T)
        xkv_t = np.ascontiguousarray(np.asarray(k_and_v[b], np.float32).T)
        in_maps.append({
            "xq8": np.ascontiguousarray(_fp8(xq_t)),
            "wq8": np.ascontiguousarray(_fp8(wq_g * WSCALE)),
            "wk8": np.ascontiguousarray(_fp8(wk_g * WSCALE)),
            "xk8": np.ascontiguousarray(_fp8(xkv_t)),
            "xkv": np.ascontiguousarray(_bf16(xkv_t)),
            "wv": np.ascontiguousarray(_bf16(wv_g)),
            "bq": np.ascontiguousarray(
                np.asarray(bq, np.float32)[hs].reshape(JC) * WSCALE),
            "bk": np.ascontiguousarray(
                np.asarray(bk, np.float32)[hs].reshape(JC) * WSCALE),
            "bv": np.ascontiguousarray(np.asarray(bv, np.float32)[hs]
                                       .reshape(JC)),
            "wo": np.ascontiguousarray(
                _bf16(wo)[g * JC:(g + 1) * JC, :]),
        })
    return in_maps


_NC_CACHE = {}


def kernel(q, k_and_v, wq, bq, wk, bk, wv, bv, wo, bo):
    if "nc" not in _NC_CACHE:
        _NC_CACHE["nc"] = build_kernel()
    nc = _NC_CACHE["nc"]
    in_maps = _prep_inputs(q, k_and_v, wq, bq, wk, bk, wv, bv, wo)
    res = run_bass_kernel_spmd(nc, in_maps, core_ids=list(range(N_CORES)))
    bo = np.asarray(bo, np.float32)
    out = np.empty((B, S, EMB), np.float32)
    for b in range(B):
        out[b] = res.results[2 * b]["out"] + res.results[2 * b + 1]["out"] + bo
    return out


# revision 38
# speedup vs baseline: 1.2543x; 1.2543x over previous
"""Multi-head self-attention TRN2 Bass kernel.

Problem: B=4, S=2048, EMB=1024, H=16 heads, dqk=dv=64.
Sharding: 8 cores; core c handles batch b=c//2 and head group g=c%2
(8 heads each). Each core computes its partial output projection
(rows of wo for its heads); host sums the two partials per batch and
adds bo.

Key optimizations over the straightforward version:
  - Q/K projections run as fp8e4 DoubleRow matmuls (256-deep
    contraction per instruction, 2x FLOP rate). Host pre-packs x/w in
    the [128p, 2k, .] interleave and scales weights+biases by 16 to
    avoid fp8 denormals; the 1/256 compensation folds into the exp
    scale (0.125/256).
  - Scores matmuls for the two heads of a pair are emitted
    interleaved at base partitions 0/64 so the K=64 matmuls row-tile
    into concurrent execution on the PE (measured ~2.1x).
  - V projection t-major with a ones column per head block so the AV
    matmul emits softmax denominators for free (row 64 of its out).
  - exp on ACT reads scores PSUM [128,1024] tiles (max-free softmax;
    |scores| small), writes bf16 P^T.
  - AV accumulates Z~T [66, 512] over 16 t-tiles in PSUM.
  - normalize: reciprocal of D row, DMA round-trip broadcast across
    partitions, multiply + bv bias -> ZnormT [512 j, 2048 s] bf16.
  - out projection bf16 -> partial OUT [2048, 1024] f32 -> DRAM.
"""

import ml_dtypes
import numpy as np

import concourse.bass as bass
import concourse.tile as tile
from concourse import bacc, mybir
from concourse.bass_utils import run_bass_kernel_spmd

B, S, EMB, H, DH = 4, 2048, 1024, 16, 64
N_CORES = 8
HPC = H // 2          # heads per core
JC = HPC * DH         # 512: per-core projected width
VB = DH + 2           # 66: per-head V block (64 V cols + ones + pad)
WSCALE = 16.0         # fp8 weight pre-scale (host); folded into exp scale

F32 = mybir.dt.float32
BF16 = mybir.dt.bfloat16
FP8 = mybir.dt.float8e4


def build_kernel(reps=1, mode='full'):
    nc = bacc.Bacc(
        "TRN2", target_bir_lowering=False, debug=False, num_devices=N_CORES
    )

    # fp8 DoubleRow operands for Q/K proj (e' = blk*256 + k*128 + p order,
    # which is just the natural row order)
    xq8 = nc.dram_tensor("xq8", [EMB, S], FP8, kind="ExternalInput").ap()
    xk8 = nc.dram_tensor("xk8", [EMB, S], FP8, kind="ExternalInput").ap()
    wq8_d = nc.dram_tensor("wq8", [EMB, JC], FP8, kind="ExternalInput").ap()
    wk8_d = nc.dram_tensor("wk8", [EMB, JC], FP8, kind="ExternalInput").ap()
    # bf16 operands for V proj
    xkv = nc.dram_tensor("xkv", [EMB, S], BF16, kind="ExternalInput").ap()
    wv_d = nc.dram_tensor("wv", [EMB, JC], BF16, kind="ExternalInput").ap()
    bq_d = nc.dram_tensor("bq", [JC], F32, kind="ExternalInput").ap()
    bk_d = nc.dram_tensor("bk", [JC], F32, kind="ExternalInput").ap()
    bv_d = nc.dram_tensor("bv", [JC], F32, kind="ExternalInput").ap()
    wo_d = nc.dram_tensor("wo", [JC, EMB], BF16, kind="ExternalInput").ap()
    out_d = nc.dram_tensor("out", [S, EMB], F32, kind="ExternalOutput").ap()
    dr_d = nc.dram_tensor("dr_scratch", [2, S], BF16).ap()  # Drecip bounce

    import contextlib

    with tile.TileContext(nc) as tc:
        with (
            tc.For_i(0, reps, 1) if reps > 1 else contextlib.nullcontext(),
            tc.tile_pool(name="persist", bufs=1) as pp,
        ):
            # persistent SBUF tensors
            qht = [pp.tile([128, S], BF16, name=f"qht{i}") for i in range(4)]
            kht = [pp.tile([128, S], BF16, name=f"kht{i}") for i in range(4)]
            vh = [pp.tile([128, HPC * VB], BF16, name=f"vh{t}")
                  for t in range(16)]
            znorm = [pp.tile([128, S], BF16, name=f"zn{i}") for i in range(4)]
            wos = [pp.tile([128, EMB], BF16, name=f"wo{j}") for j in range(4)]
            bias_q = pp.tile([128, 4], F32, name="bias_q")
            bias_k = pp.tile([128, 4], F32, name="bias_k")
            bias_v = pp.tile([64, HPC], F32, name="bias_v")  # [d, head]

            nc.sync.dma_start(bias_q[:], bq_d.rearrange("(c p) -> p c", p=128))
            nc.sync.dma_start(bias_k[:], bk_d.rearrange("(c p) -> p c", p=128))
            # ones columns in vh blocks (col 64 of each 66-block); pad col 0
            for t in range(16):
                blocks = vh[t][:].rearrange("p (h c) -> p h c", c=VB)
                nc.vector.memset(blocks[:, :, DH:DH + 1], 1.0)
                nc.vector.memset(blocks[:, :, DH + 1:], 0.0)

            # ------------- fused projections + attention -------------
            # pair-0 QK proj gates the first exp; V-proj uses the av PSUM
            # slots so pair-0 scores/exp (sp slots) start immediately
            # after. Pairs 1-3 QK proj is emitted inside the previous
            # pair's attention t-loop so its sp-ring slots interleave
            # with the scores pipeline instead of all queuing ahead of
            # it (tile-tag rings are ordered by emission).
            with (
                tc.tile_pool(name="sps", bufs=1, space="PSUM") as sp_pool,
                tc.tile_pool(name="avps", bufs=1, space="PSUM") as av_pool,
                tc.tile_pool(name="dreb_p", bufs=2) as dre_pool,
                tc.tile_pool(name="znsc", bufs=1) as zns_pool,
                tc.tile_pool(name="drec_p", bufs=1) as drec_pool,
                tc.tile_pool(name="avstg", bufs=2) as stg_pool,
                tc.tile_pool(name="xr_p", bufs=1) as xr_pool,
                tc.tile_pool(name="wqkv", bufs=1) as w_pool,
            ):
                # fp8 DoubleRow tiles: [128p, 2k, .] with k (the 128-row
                # subtile index) major in the free dim
                wq8 = [w_pool.tile([128, 2 * JC], FP8, name=f"wq8_{e}")
                       for e in range(4)]
                wk8 = [w_pool.tile([128, 2 * JC], FP8, name=f"wk8_{e}")
                       for e in range(4)]
                wvs = [w_pool.tile([128, JC], BF16, name=f"wvs{e}")
                       for e in range(8)]
                xq8r = [xr_pool.tile([128, 2 * S], FP8, name=f"xq8r{e}")
                        for e in range(4)]
                xk8r = [xr_pool.tile([128, 2 * S], FP8, name=f"xk8r{e}")
                        for e in range(4)]
                xkr = [xr_pool.tile([128, S], BF16, name=f"xkr{e}")
                       for e in range(8)]
                # DMA in first-use order: Q/K fp8 path first (gates the
                # first exp), then the V-proj operands, then wo/bias_v
                # (needed last).
                for e in range(4):
                    for k in range(2):
                        r0 = e * 256 + k * 128
                        nc.sync.dma_start(
                            wq8[e][:, k * JC:(k + 1) * JC],
                            wq8_d[r0:r0 + 128, :])
                half = S // 2
                for e in range(4):
                    for k in range(2):
                        r0 = e * 256 + k * 128
                        nc.sync.dma_start(
                            xq8r[e][:, k * S:k * S + half],
                            xq8[r0:r0 + 128, 0:half])
                for e in range(4):
                    for k in range(2):
                        r0 = e * 256 + k * 128
                        nc.sync.dma_start(
                            wk8[e][:, k * JC:(k + 1) * JC],
                            wk8_d[r0:r0 + 128, :])
                for e in range(4):
                    for k in range(2):
                        r0 = e * 256 + k * 128
                        nc.sync.dma_start(
                            xk8r[e][:, k * S:k * S + half],
                            xk8[r0:r0 + 128, 0:half])
                for xr_t, xd in ((xq8r, xq8), (xk8r, xk8)):
                    for e in range(4):
                        for k in range(2):
                            r0 = e * 256 + k * 128
                            nc.sync.dma_start(
                                xr_t[e][:, k * S + half:(k + 1) * S],
                                xd[r0:r0 + 128, half:S])
                for e in range(8):
                    nc.sync.dma_start(wvs[e][:],
                                      wv_d[e * 128:(e + 1) * 128, :])
                for e in range(8):
                    nc.sync.dma_start(xkr[e][:],
                                      xkv[e * 128:(e + 1) * 128, :])
                nc.sync.dma_start(bias_v[:],
                                  bv_d.rearrange("(h d) -> d h", d=DH))
                for j in range(4):
                    nc.sync.dma_start(wos[j][:],
                                      wo_d[j * 128:(j + 1) * 128, :])

                # sc-interleaved so scores/exp for s_half 0 can start
                # after Q sc0,sc1 + K sc0
                PROJ_ORDER = (('q', 0), ('q', 1), ('k', 0), ('k', 1),
                              ('q', 2), ('k', 2), ('q', 3), ('k', 3))

                def proj_one(pair, which, sc, av_tag=None):
                    jsl = slice(pair * 128, (pair + 1) * 128)
                    dst, ws, xr, bias = (
                        (qht, wq8, xq8r, bias_q) if which == 'q'
                        else (kht, wk8, xk8r, bias_k))
                    s0 = sc * 512
                    if av_tag is None:
                        ps = sp_pool.tile([128, 512], F32, tag=f"sp{sc % 2}",
                                          name=f"sp{sc % 2}")
                    else:
                        # av-ring slot: the pt ring buffers the exp stream
                        # while the next s_half's AVs wait on this slot, so
                        # (unlike the sp ring) this costs no exp bubbles
                        ps = av_pool.tile([128, 512], F32, tag=av_tag,
                                          name=av_tag)
                    for e in range(4):
                        w3 = ws[e][:].rearrange("p (k j) -> p k j", k=2)
                        x3 = xr[e][:].rearrange("p (k s) -> p k s", k=2)
                        nc.tensor.matmul(
                            ps[:],
                            w3[:, :, jsl],
                            x3[:, :, s0:s0 + 512],
                            start=(e == 0), stop=(e == 3),
                            perf_mode=mybir.MatmulPerfMode.DoubleRow,
                        )
                    nc.vector.tensor_scalar_add(
                        dst[pair][:, s0:s0 + 512], ps[:],
                        bias[:, pair:pair + 1]
                    )

                def emit_vproj():
                    # V-proj goes through the av PSUM slots (free until
                    # the first AV accumulation) so it can overlap with
                    # pair-0 scores+exp, which use the sp slots.
                    for tch in range(16):
                        tsl = slice(tch * 128, (tch + 1) * 128)
                        vt = tch % 4
                        ps = av_pool.tile(
                            [128, 512], F32,
                            tag=f"av{vt // 2}{vt % 2}",
                            name=f"av{vt // 2}{vt % 2}")
                        for e in range(8):
                            nc.tensor.matmul(
                                ps[:], xkr[e][:, tsl], wvs[e][:],
                                start=(e == 0), stop=(e == 7),
                            )
                        nc.vector.tensor_copy(
                            vh[tch][:].rearrange(
                                "p (h c) -> p h c", c=VB)[:, :, 0:DH],
                            ps[:].rearrange("p (h d) -> p h d", d=DH),
                        )

                def emit_attn(pair, pt_pool, proj_units=()):
                    for s_half in range(2):
                        s0 = s_half * 1024
                        avs = {}
                        for par in range(2):
                            for i in range(2):
                                avs[(par, i)] = av_pool.tile(
                                    [VB, 512], F32, tag=f"av{par}{i}",
                                    name=f"av{par}{i}")

                        def emit_scores(t):
                            """Both heads' score matmuls interleaved so the
                            K=64 pairs row-tile concurrently (base
                            partitions 0 / 64)."""
                            sps = []
                            for par in range(2):
                                sps.append(sp_pool.tile(
                                    [128, 1024], F32, tag=f"sp{par}",
                                    name=f"sp{par}"))
                            for rep in range(2 if mode == 'sc2' else 1):
                                for i in range(2):
                                    for par in range(2):
                                        off = par * 64
                                        nc.tensor.matmul(
                                            sps[par][:,
                                                     i * 512:(i + 1) * 512],
                                            kht[pair][off:off + 64,
                                                      t * 128:(t + 1) * 128],
                                            qht[pair][off:off + 64,
                                                      s0 + i * 512:
                                                      s0 + (i + 1) * 512],
                                            start=True,
                                            stop=True,
                                        )
                            ptts = []
                            for par in range(2):
                                ptt = pt_pool.tile([128, 1024], BF16,
                                                   tag=f"ptt{par}",
                                                   name=f"ptt{par}")
                                nc.scalar.activation(
                                    ptt[:], sps[par][:],
                                    mybir.ActivationFunctionType.Exp,
                                    scale=0.125 / (WSCALE * WSCALE),
                                )
                                if mode == 'exp2':
                                    pt2 = pt_pool.tile(
                                        [128, 1024], BF16,
                                        tag=f"pt2{par}", name=f"pt2{par}")
                                    nc.scalar.activation(
                                        pt2[:], sps[par][:],
                                        mybir.ActivationFunctionType.Exp,
                                        scale=0.125 / (WSCALE * WSCALE),
                                    )
                                ptts.append(ptt)
                            return ptts

                        def emit_av(t, par, ptt):
                            h = pair * 2 + par
                            for i in range(2):
                                for rep in range(2 if mode == 'av2' else 1):
                                    nc.tensor.matmul(
                                        avs[(par, i)][:],
                                        vh[t][:, h * VB:(h + 1) * VB],
                                        ptt[:, i * 512:(i + 1) * 512],
                                        start=(t == 0 and rep == 0),
                                        stop=(t == 15 and
                                              rep == (1 if mode == 'av2'
                                                      else 0)),
                                        skip_group_check=True,
                                    )

                        prev = None
                        for t in range(16):
                            ptts = emit_scores(t)
                            if prev is not None:
                                emit_av(prev[0], 0, prev[1])
                                emit_av(prev[0], 1, prev[2])
                            prev = (t, ptts[0], ptts[1])
                        emit_av(prev[0], 0, prev[1])
                        emit_av(prev[0], 1, prev[2])

                        # stage avs out to SBUF fast so the next s_half's
                        # AV t=0 only waits on these copies, not on the
                        # full normalize chain
                        stg = {}
                        for par in range(2):
                            for i in range(2):
                                stg[(par, i)] = stg_pool.tile(
                                    [66, 512], BF16, tag=f"avs{par}{i}",
                                    name=f"avs{par}{i}")
                                nc.vector.tensor_copy(
                                    stg[(par, i)][:], avs[(par, i)][:])
                        # pairs 1-3 QK proj rides the av ring at s_half
                        # boundaries: p1 all 8 units at the first boundary,
                        # later pairs 4 per boundary, always finishing
                        # before their own attention starts
                        n_units = 8 if (pair, s_half) == (0, 0) else 4
                        for u in range(n_units):
                            if not proj_units:
                                break
                            p_, w_, sc_ = proj_units.pop(0)
                            proj_one(p_, w_, sc_, av_tag=f"av{u % 4 // 2}{u % 2}")
                        for par in range(2):
                            h = pair * 2 + par
                            off = par * 64
                            dreb = dre_pool.tile(
                                [64, 1024], BF16, tag=f"dreb{par}",
                                name=f"dreb{par}")
                            for i in range(2):
                                drc = drec_pool.tile(
                                    [1, 512], BF16, tag=f"drc{par}",
                                    name=f"drc{par}")
                                with nc.allow_low_precision(
                                        reason="1/D in bf16: recip err "
                                        "~0.2% is shared per-head and "
                                        "attenuates to ~3e-5 in out"):
                                    nc.vector.reciprocal(
                                        drc[0:1, :],
                                        stg[(par, i)][DH:DH + 1, :],
                                    )
                                nc.sync.dma_start(
                                    dr_d[par,
                                         s0 + i * 512:s0 + (i + 1) * 512],
                                    drc[0:1, :],
                                )
                            nc.sync.dma_start(
                                dreb[:],
                                dr_d[par:par + 1, s0:s0 + 1024]
                                .broadcast_to([64, 1024]),
                            )
                            zn_s = zns_pool.tile(
                                [64, 1024], BF16, tag=f"zn_s{par}",
                                name=f"zn_s{par}")
                            for i in range(2):
                                nc.vector.tensor_mul(
                                    zn_s[:, i * 512:(i + 1) * 512],
                                    stg[(par, i)][0:DH, :],
                                    dreb[:, i * 512:(i + 1) * 512],
                                )
                            nc.vector.tensor_scalar_add(
                                zn_s[:], zn_s[:], bias_v[:, h:h + 1]
                            )
                            nc.sync.dma_start(
                                znorm[pair][off:off + 64, s0:s0 + 1024],
                                zn_s[:],
                            )

                with tc.tile_pool(name="pt", bufs=8) as pt_pool:
                    n_pairs = {'pairs1': 1, 'pairs0': 0}.get(mode, 4)
                    for p_ in range(n_pairs, 4):
                        nc.vector.memset(znorm[p_][:], 0.0)
                    for idx in range(8):
                        proj_one(0, *PROJ_ORDER[idx])
                    emit_vproj()
                    proj_units = [(p, w, sc) for p in range(1, 4)
                                  for (w, sc) in PROJ_ORDER]
                    for pair in range(n_pairs):
                        emit_attn(pair, pt_pool, proj_units=proj_units)

            # ---------------- output projection ----------------
            with (
                tc.tile_pool(name="ops", bufs=4, space="PSUM") as op_pool,
                tc.tile_pool(name="ostg", bufs=4) as ostg_pool,
            ):
                for scc in range(16):
                    psl = slice(scc * 128, (scc + 1) * 128)
                    for oc in range(2):
                        ps = op_pool.tile([128, 512], F32, tag="ops",
                                          name="ops")
                        osl = slice(oc * 512, (oc + 1) * 512)
                        for jt in range(4):
                            nc.tensor.matmul(
                                ps[:],
                                znorm[jt][:, psl],
                                wos[jt][:, osl],
                                start=(jt == 0),
                                stop=(jt == 3),
                            )
                        ostg = ostg_pool.tile([128, 512], F32, tag="ostg",
                                              name="ostg")
                        nc.vector.tensor_copy(ostg[:], ps[:])
                        nc.sync.dma_start(out_d[psl, osl], ostg[:])

    nc.compile()
    return nc


def _bf16(a):
    return np.asarray(a, np.float32).astype(ml_dtypes.bfloat16)


def _fp8(a):
    return np.asarray(a, np.float32).astype(ml_dtypes.float8_e4m3fn)


def _prep_inputs(q, k_and_v, wq, bq, wk, bk, wv, bv, wo):
    """Build per-core input maps."""
    in_maps = []
    for c in range(N_CORES):
        b, g = c // 2, c % 2
        hs = slice(g * HPC, (g + 1) * HPC)
        # [H, emb, d] -> [emb, H*d] for this head group
        wq_g = np.transpose(wq[hs], (1, 0, 2)).reshape(EMB, JC)
        wk_g = np.transpose(wk[hs], (1, 0, 2)).reshape(EMB, JC)
        wv_g = np.transpose(wv[hs], (1, 0, 2)).reshape(EMB, JC)
        xq_t = np.ascontiguousarray(np.asarray(q[b], np.float32).T)
        xkv_t = np.ascontiguousarray(np.asarray(k_and_v[b], np.float32).T)
        in_maps.append({
            "xq8": np.ascontiguousarray(_fp8(xq_t)),
            "wq8": np.ascontiguousarray(_fp8(wq_g * WSCALE)),
            "wk8": np.ascontiguousarray(_fp8(wk_g * WSCALE)),
            "xk8": np.ascontiguousarray(_fp8(xkv_t)),
            "xkv": np.ascontiguousarray(_bf16(xkv_t)),
            "wv": np.ascontiguousarray(_bf16(wv_g)),
            "bq": np.ascontiguousarray(
                np.asarray(bq, np.float32)[hs].reshape(JC) * WSCALE),
            "bk": np.ascontiguousarray(
                np.asarray(bk, np.float32)[hs].reshape(JC) * WSCALE),
            "bv": np.ascontiguousarray(np.asarray(bv, np.float32)[hs]
                                       .reshape(JC)),
            "wo": np.ascontiguousarray(
                _bf16(wo)[g * JC:(g + 1) * JC, :]),
        })
    return in_maps


_NC_CACHE = {}


def kernel(q, k_and_v, wq, bq, wk, bk, wv, bv, wo, bo):
    if "nc" not in _NC_CACHE:
        _NC_CACHE["nc"] = build_kernel()
    nc = _NC_CACHE["nc"]
    in_maps = _prep_inputs(q, k_and_v, wq, bq, wk, bk, wv, bv, wo)
    res = run_bass_kernel_spmd(nc, in_maps, core_ids=list(range(N_CORES)))
    bo = np.asarray(bo, np.float32)
    out = np.empty((B, S, EMB), np.float32)
    for b in range(B):
        out[b] = res.results[2 * b]["out"] + res.results[2 * b + 1]["out"] + bo
    return out


# revision 40
# speedup vs baseline: 1.3192x; 1.0517x over previous
"""Multi-head self-attention TRN2 Bass kernel.

Problem: B=4, S=2048, EMB=1024, H=16 heads, dqk=dv=64.
Sharding: 8 cores; core c handles batch b=c//2 and head group g=c%2
(8 heads each). Each core computes its partial output projection
(rows of wo for its heads); host sums the two partials per batch and
adds bo.

Key optimizations over the straightforward version:
  - Q/K projections run as fp8e4 DoubleRow matmuls (256-deep
    contraction per instruction, 2x FLOP rate). Host pre-packs x/w in
    the [128p, 2k, .] interleave and scales weights+biases by 16 to
    avoid fp8 denormals; the 1/256 compensation folds into the exp
    scale (0.125/256).
  - Scores matmuls for the two heads of a pair are emitted
    interleaved at base partitions 0/64 so the K=64 matmuls row-tile
    into concurrent execution on the PE (measured ~2.1x).
  - V projection t-major with a ones column per head block so the AV
    matmul emits softmax denominators for free (row 64 of its out).
  - exp on ACT reads scores PSUM [128,1024] tiles (max-free softmax;
    |scores| small), writes bf16 P^T.
  - AV accumulates Z~T [66, 512] over 16 t-tiles in PSUM.
  - normalize: reciprocal of D row, DMA round-trip broadcast across
    partitions, multiply + bv bias -> ZnormT [512 j, 2048 s] bf16.
  - out projection bf16 -> partial OUT [2048, 1024] f32 -> DRAM.
"""

import ml_dtypes
import numpy as np

import concourse.bass as bass
import concourse.tile as tile
from concourse import bacc, mybir
from concourse.bass_utils import run_bass_kernel_spmd

B, S, EMB, H, DH = 4, 2048, 1024, 16, 64
N_CORES = 8
HPC = H // 2          # heads per core
JC = HPC * DH         # 512: per-core projected width
VB = DH + 2           # 66: per-head V block (64 V cols + ones + pad)
WSCALE = 16.0         # fp8 weight pre-scale (host); folded into exp scale

F32 = mybir.dt.float32
BF16 = mybir.dt.bfloat16
FP8 = mybir.dt.float8e4


def build_kernel(reps=1, mode='full'):
    nc = bacc.Bacc(
        "TRN2", target_bir_lowering=False, debug=False, num_devices=N_CORES
    )

    # fp8 DoubleRow operands for Q/K proj (e' = blk*256 + k*128 + p order,
    # which is just the natural row order)
    xq8 = nc.dram_tensor("xq8", [EMB, S], FP8, kind="ExternalInput").ap()
    xk8 = nc.dram_tensor("xk8", [EMB, S], FP8, kind="ExternalInput").ap()
    wq8_d = nc.dram_tensor("wq8", [EMB, JC], FP8, kind="ExternalInput").ap()
    wk8_d = nc.dram_tensor("wk8", [EMB, JC], FP8, kind="ExternalInput").ap()
    # bf16 operands for V proj
    xkv = nc.dram_tensor("xkv", [EMB, S], BF16, kind="ExternalInput").ap()
    wv_d = nc.dram_tensor("wv", [EMB, JC], BF16, kind="ExternalInput").ap()
    bq_d = nc.dram_tensor("bq", [JC], F32, kind="ExternalInput").ap()
    bk_d = nc.dram_tensor("bk", [JC], F32, kind="ExternalInput").ap()
    bv_d = nc.dram_tensor("bv", [JC], F32, kind="ExternalInput").ap()
    wo_d = nc.dram_tensor("wo", [JC, EMB], BF16, kind="ExternalInput").ap()
    out_d = nc.dram_tensor("out", [S, EMB], F32, kind="ExternalOutput").ap()
    dr_d = nc.dram_tensor("dr_scratch", [2, S], BF16).ap()  # Drecip bounce

    import contextlib

    with tile.TileContext(nc) as tc:
        with (
            tc.For_i(0, reps, 1) if reps > 1 else contextlib.nullcontext(),
            tc.tile_pool(name="persist", bufs=1) as pp,
        ):
            # persistent SBUF tensors
            qht = [pp.tile([128, S], BF16, name=f"qht{i}") for i in range(4)]
            kht = [pp.tile([128, S], BF16, name=f"kht{i}") for i in range(4)]
            vh = [pp.tile([128, HPC * VB], BF16, name=f"vh{t}")
                  for t in range(16)]
            znorm = [pp.tile([128, S], BF16, name=f"zn{i}") for i in range(4)]
            wos = [pp.tile([128, EMB], BF16, name=f"wo{j}") for j in range(4)]
            bias_q = pp.tile([128, 4], F32, name="bias_q")
            bias_k = pp.tile([128, 4], F32, name="bias_k")
            bias_v = pp.tile([64, HPC], F32, name="bias_v")  # [d, head]

            nc.sync.dma_start(bias_q[:], bq_d.rearrange("(c p) -> p c", p=128))
            nc.sync.dma_start(bias_k[:], bk_d.rearrange("(c p) -> p c", p=128))
            # ones columns in vh blocks (col 64 of each 66-block); pad col 0
            for t in range(16):
                blocks = vh[t][:].rearrange("p (h c) -> p h c", c=VB)
                nc.vector.memset(blocks[:, :, DH:DH + 1], 1.0)
                nc.vector.memset(blocks[:, :, DH + 1:], 0.0)

            # ------------- fused projections + attention -------------
            # pair-0 QK proj gates the first exp; V-proj uses the av PSUM
            # slots so pair-0 scores/exp (sp slots) start immediately
            # after. Pairs 1-3 QK proj is emitted inside the previous
            # pair's attention t-loop so its sp-ring slots interleave
            # with the scores pipeline instead of all queuing ahead of
            # it (tile-tag rings are ordered by emission).
            with (
                tc.tile_pool(name="sps", bufs=1, space="PSUM") as sp_pool,
                tc.tile_pool(name="avps", bufs=1, space="PSUM") as av_pool,
                tc.tile_pool(name="dreb_p", bufs=2) as dre_pool,
                tc.tile_pool(name="znsc", bufs=1) as zns_pool,
                tc.tile_pool(name="drec_p", bufs=1) as drec_pool,
                tc.tile_pool(name="avstg", bufs=2) as stg_pool,
                tc.tile_pool(name="xr_p", bufs=1) as xr_pool,
                tc.tile_pool(name="wqkv", bufs=1) as w_pool,
            ):
                # fp8 DoubleRow tiles: [128p, 2k, .] with k (the 128-row
                # subtile index) major in the free dim
                wq8 = [w_pool.tile([128, 2 * JC], FP8, name=f"wq8_{e}")
                       for e in range(4)]
                wk8 = [w_pool.tile([128, 2 * JC], FP8, name=f"wk8_{e}")
                       for e in range(4)]
                wvs = [w_pool.tile([128, JC], BF16, name=f"wvs{e}")
                       for e in range(8)]
                xq8r = [xr_pool.tile([128, 2 * S], FP8, name=f"xq8r{e}")
                        for e in range(4)]
                xk8r = [xr_pool.tile([128, 2 * S], FP8, name=f"xk8r{e}")
                        for e in range(4)]
                xkr = [xr_pool.tile([128, S], BF16, name=f"xkr{e}")
                       for e in range(8)]
                # DMA in first-use order: Q/K fp8 path first (gates the
                # first exp), then the V-proj operands, then wo/bias_v
                # (needed last).
                for e in range(4):
                    for k in range(2):
                        r0 = e * 256 + k * 128
                        nc.sync.dma_start(
                            wq8[e][:, k * JC:(k + 1) * JC],
                            wq8_d[r0:r0 + 128, :])
                half = S // 2
                for e in range(4):
                    for k in range(2):
                        r0 = e * 256 + k * 128
                        nc.sync.dma_start(
                            xq8r[e][:, k * S:k * S + half],
                            xq8[r0:r0 + 128, 0:half])
                for e in range(4):
                    for k in range(2):
                        r0 = e * 256 + k * 128
                        nc.sync.dma_start(
                            wk8[e][:, k * JC:(k + 1) * JC],
                            wk8_d[r0:r0 + 128, :])
                for e in range(4):
                    for k in range(2):
                        r0 = e * 256 + k * 128
                        nc.sync.dma_start(
                            xk8r[e][:, k * S:k * S + half],
                            xk8[r0:r0 + 128, 0:half])
                for xr_t, xd in ((xq8r, xq8), (xk8r, xk8)):
                    for e in range(4):
                        for k in range(2):
                            r0 = e * 256 + k * 128
                            nc.sync.dma_start(
                                xr_t[e][:, k * S + half:(k + 1) * S],
                                xd[r0:r0 + 128, half:S])
                for e in range(8):
                    nc.sync.dma_start(wvs[e][:],
                                      wv_d[e * 128:(e + 1) * 128, :])
                for e in range(8):
                    nc.sync.dma_start(xkr[e][:],
                                      xkv[e * 128:(e + 1) * 128, :])
                nc.sync.dma_start(bias_v[:],
                                  bv_d.rearrange("(h d) -> d h", d=DH))
                for j in range(4):
                    nc.sync.dma_start(wos[j][:],
                                      wo_d[j * 128:(j + 1) * 128, :])

                # sc-interleaved so scores/exp for s_half 0 can start
                # after Q sc0,sc1 + K sc0
                PROJ_ORDER = (('q', 0), ('q', 1), ('k', 0), ('k', 1),
                              ('q', 2), ('k', 2), ('q', 3), ('k', 3))

                def proj_one(pair, which, sc, av_tag=None):
                    jsl = slice(pair * 128, (pair + 1) * 128)
                    dst, ws, xr, bias = (
                        (qht, wq8, xq8r, bias_q) if which == 'q'
                        else (kht, wk8, xk8r, bias_k))
                    s0 = sc * 512
                    if av_tag is None:
                        ps = sp_pool.tile([128, 512], F32, tag=f"sp{sc % 2}",
                                          name=f"sp{sc % 2}")
                    else:
                        # av-ring slot: the pt ring buffers the exp stream
                        # while the next s_half's AVs wait on this slot, so
                        # (unlike the sp ring) this costs no exp bubbles
                        ps = av_pool.tile([128, 512], F32, tag=av_tag,
                                          name=av_tag)
                    for e in range(4):
                        w3 = ws[e][:].rearrange("p (k j) -> p k j", k=2)
                        x3 = xr[e][:].rearrange("p (k s) -> p k s", k=2)
                        nc.tensor.matmul(
                            ps[:],
                            w3[:, :, jsl],
                            x3[:, :, s0:s0 + 512],
                            start=(e == 0), stop=(e == 3),
                            perf_mode=mybir.MatmulPerfMode.DoubleRow,
                        )
                    nc.vector.tensor_scalar_add(
                        dst[pair][:, s0:s0 + 512], ps[:],
                        bias[:, pair:pair + 1]
                    )

                def emit_vproj():
                    # V-proj goes through the av PSUM slots (free until
                    # the first AV accumulation) so it can overlap with
                    # pair-0 scores+exp, which use the sp slots.
                    for tch in range(16):
                        tsl = slice(tch * 128, (tch + 1) * 128)
                        vt = tch % 4
                        ps = av_pool.tile(
                            [128, 512], F32,
                            tag=f"av{vt // 2}{vt % 2}",
                            name=f"av{vt // 2}{vt % 2}")
                        for e in range(8):
                            nc.tensor.matmul(
                                ps[:], xkr[e][:, tsl], wvs[e][:],
                                start=(e == 0), stop=(e == 7),
                            )
                        nc.vector.tensor_copy(
                            vh[tch][:].rearrange(
                                "p (h c) -> p h c", c=VB)[:, :, 0:DH],
                            ps[:].rearrange("p (h d) -> p h d", d=DH),
                        )

                def emit_attn(pair, pt_pool, proj_units=()):
                    for s_half in range(2):
                        s0 = s_half * 1024
                        avs = {}
                        for par in range(2):
                            for i in range(2):
                                avs[(par, i)] = av_pool.tile(
                                    [VB, 512], F32, tag=f"av{par}{i}",
                                    name=f"av{par}{i}")

                        def emit_scores(t):
                            """Both heads' score matmuls interleaved so the
                            K=64 pairs row-tile concurrently (base
                            partitions 0 / 64)."""
                            sps = []
                            for par in range(2):
                                sps.append(sp_pool.tile(
                                    [128, 1024], F32, tag=f"sp{par}",
                                    name=f"sp{par}"))
                            for rep in range(2 if mode == 'sc2' else 1):
                                for i in range(2):
                                    for par in range(2):
                                        off = par * 64
                                        nc.tensor.matmul(
                                            sps[par][:,
                                                     i * 512:(i + 1) * 512],
                                            kht[pair][off:off + 64,
                                                      t * 128:(t + 1) * 128],
                                            qht[pair][off:off + 64,
                                                      s0 + i * 512:
                                                      s0 + (i + 1) * 512],
                                            start=True,
                                            stop=True,
                                        )
                            ptts = []
                            for par in range(2):
                                ptt = pt_pool.tile([128, 1024], BF16,
                                                   tag=f"ptt{par}",
                                                   name=f"ptt{par}")
                                nc.scalar.activation(
                                    ptt[:], sps[par][:],
                                    mybir.ActivationFunctionType.Exp,
                                    scale=0.125 / (WSCALE * WSCALE),
                                )
                                if mode == 'exp2':
                                    pt2 = pt_pool.tile(
                                        [128, 1024], BF16,
                                        tag=f"pt2{par}", name=f"pt2{par}")
                                    nc.scalar.activation(
                                        pt2[:], sps[par][:],
                                        mybir.ActivationFunctionType.Exp,
                                        scale=0.125 / (WSCALE * WSCALE),
                                    )
                                ptts.append(ptt)
                            return ptts

                        def emit_av(t, par, ptt):
                            h = pair * 2 + par
                            for i in range(2):
                                for rep in range(2 if mode == 'av2' else 1):
                                    nc.tensor.matmul(
                                        avs[(par, i)][:],
                                        vh[t][:, h * VB:(h + 1) * VB],
                                        ptt[:, i * 512:(i + 1) * 512],
                                        start=(t == 0 and rep == 0),
                                        stop=(t == 15 and
                                              rep == (1 if mode == 'av2'
                                                      else 0)),
                                        skip_group_check=True,
                                    )

                        prev = None
                        for t in range(16):
                            ptts = emit_scores(t)
                            if prev is not None:
                                emit_av(prev[0], 0, prev[1])
                                emit_av(prev[0], 1, prev[2])
                            prev = (t, ptts[0], ptts[1])
                        emit_av(prev[0], 0, prev[1])
                        emit_av(prev[0], 1, prev[2])

                        # stage avs out to SBUF fast so the next s_half's
                        # AV t=0 only waits on these copies, not on the
                        # full normalize chain
                        stg = {}
                        for par in range(2):
                            for i in range(2):
                                stg[(par, i)] = stg_pool.tile(
                                    [66, 512], BF16, tag=f"avs{par}{i}",
                                    name=f"avs{par}{i}")
                                nc.vector.tensor_copy(
                                    stg[(par, i)][:], avs[(par, i)][:])
                        # pairs 1-3 QK proj rides the av ring at s_half
                        # boundaries: p1 all 8 units at the first boundary,
                        # later pairs 4 per boundary, always finishing
                        # before their own attention starts
                        n_units = 8 if (pair, s_half) == (0, 0) else 4
                        for u in range(n_units):
                            if not proj_units:
                                break
                            p_, w_, sc_ = proj_units.pop(0)
                            proj_one(p_, w_, sc_, av_tag=f"av{u % 4 // 2}{u % 2}")
                        for par in range(2):
                            h = pair * 2 + par
                            off = par * 64
                            dreb = dre_pool.tile(
                                [64, 1024], BF16, tag=f"dreb{par}",
                                name=f"dreb{par}")
                            for i in range(2):
                                drc = drec_pool.tile(
                                    [1, 512], BF16, tag=f"drc{par}",
                                    name=f"drc{par}")
                                with nc.allow_low_precision(
                                        reason="1/D in bf16: recip err "
                                        "~0.2% is shared per-head and "
                                        "attenuates to ~3e-5 in out"):
                                    nc.vector.reciprocal(
                                        drc[0:1, :],
                                        stg[(par, i)][DH:DH + 1, :],
                                    )
                                nc.sync.dma_start(
                                    dr_d[par,
                                         s0 + i * 512:s0 + (i + 1) * 512],
                                    drc[0:1, :],
                                )
                            nc.sync.dma_start(
                                dreb[:],
                                dr_d[par:par + 1, s0:s0 + 1024]
                                .broadcast_to([64, 1024]),
                            )
                            zn_s = zns_pool.tile(
                                [64, 1024], BF16, tag=f"zn_s{par}",
                                name=f"zn_s{par}")
                            for i in range(2):
                                nc.vector.tensor_mul(
                                    zn_s[:, i * 512:(i + 1) * 512],
                                    stg[(par, i)][0:DH, :],
                                    dreb[:, i * 512:(i + 1) * 512],
                                )
                            nc.vector.tensor_scalar_add(
                                zn_s[:], zn_s[:], bias_v[:, h:h + 1]
                            )
                            nc.sync.dma_start(
                                znorm[pair][off:off + 64, s0:s0 + 1024],
                                zn_s[:],
                            )

                with tc.tile_pool(name="pt", bufs=8) as pt_pool:
                    n_pairs = {'pairs1': 1, 'pairs0': 0}.get(mode, 4)
                    for p_ in range(n_pairs, 4):
                        nc.vector.memset(znorm[p_][:], 0.0)
                    for idx in range(8):
                        proj_one(0, *PROJ_ORDER[idx])
                    emit_vproj()
                    proj_units = [(p, w, sc) for p in range(1, 4)
                                  for (w, sc) in PROJ_ORDER]
                    for pair in range(n_pairs):
                        emit_attn(pair, pt_pool, proj_units=proj_units)

            # ---------------- output projection ----------------
            with (
                tc.tile_pool(name="ops", bufs=4, space="PSUM") as op_pool,
                tc.tile_pool(name="ostg", bufs=4) as ostg_pool,
            ):
                for scc in range(16):
                    psl = slice(scc * 128, (scc + 1) * 128)
                    for oc in range(2):
                        ps = op_pool.tile([128, 512], F32, tag="ops",
                                          name="ops")
                        osl = slice(oc * 512, (oc + 1) * 512)
                        for jt in range(4):
                            nc.tensor.matmul(
                                ps[:],
                                znorm[jt][:, psl],
                                wos[jt][:, osl],
                                start=(jt == 0),
                                stop=(jt == 3),
                            )
                        ostg = ostg_pool.tile([128, 512], F32, tag="ostg",
                                              name="ostg")
                        nc.vector.tensor_copy(ostg[:], ps[:])
                        nc.sync.dma_start(out_d[psl, osl], ostg[:])

    nc.compile()
    return nc


def _bf16(a):
    return np.asarray(a, np.float32).astype(ml_dtypes.bfloat16)


def _fp8(a):
    return np.asarray(a, np.float32).astype(ml_dtypes.float8_e4m3fn)


def _prep_inputs(q, k_and_v, wq, bq, wk, bk, wv, bv, wo):
    """Build per-core input maps."""
    in_maps = []
    for c in range(N_CORES):
        b, g = c // 2, c % 2
        hs = slice(g * HPC, (g + 1) * HPC)
        # [H, emb, d] -> [emb, H*d] for this head group
        wq_g = np.transpose(wq[hs], (1, 0, 2)).reshape(EMB, JC)
        wk_g = np.transpose(wk[hs], (1, 0, 2)).reshape(EMB, JC)
        wv_g = np.transpose(wv[hs], (1, 0, 2)).reshape(EMB, JC)
        xq_t = np.ascontiguousarray(np.asarray(q[b], np.float32).T)
        xkv_t = np.ascontiguousarray(np.asarray(k_and_v[b], np.float32).T)
        in_maps.append({
            "xq8": np.ascontiguousarray(_fp8(xq_t)),
            "wq8": np.ascontiguousarray(_fp8(wq_g * WSCALE)),
            "wk8": np.ascontiguousarray(_fp8(wk_g * WSCALE)),
            "xk8": np.ascontiguousarray(_fp8(xkv_t)),
            "xkv": np.ascontiguousarray(_bf16(xkv_t)),
            "wv": np.ascontiguousarray(_bf16(wv_g)),
            "bq": np.ascontiguousarray(
                np.asarray(bq, np.float32)[hs].reshape(JC) * WSCALE),
            "bk": np.ascontiguousarray(
                np.asarray(bk, np.float32)[hs].reshape(JC) * WSCALE),
            "bv": np.ascontiguousarray(np.asarray(bv, np.float32)[hs]
                                       .reshape(JC)),
            "wo": np.ascontiguousarray(
                _bf16(wo)[g * JC:(g + 1) * JC, :]),
        })
    return in_maps


_NC_CACHE = {}


def kernel(q, k_and_v, wq, bq, wk, bk, wv, bv, wo, bo):
    if "nc" not in _NC_CACHE:
        _NC_CACHE["nc"] = build_kernel()
    nc = _NC_CACHE["nc"]
    in_maps = _prep_inputs(q, k_and_v, wq, bq, wk, bk, wv, bv, wo)
    res = run_bass_kernel_spmd(nc, in_maps, core_ids=list(range(N_CORES)))
    bo = np.asarray(bo, np.float32)
    out = np.empty((B, S, EMB), np.float32)
    for b in range(B):
        out[b] = res.results[2 * b]["out"] + res.results[2 * b + 1]["out"] + bo
    return out
